# revision 6
# baseline (speedup 1.0000x reference)
"""Trainium2 Bass kernel for EquivariantLieConvLayer (GNN message passing).

Math restructuring (exact algebra, not approximation):
  reference computes, per edge e = (s -> t):
      msg_e = alpha_bil * bracket(alpha_msg * F[s], F[t])
      agg[t] += msg_e
      out = F + agg + update_scale * bracket(agg, alpha_w * agg)
  * bracket is bilinear and F[t] is shared by all edges targeting t, so
      agg[t] = alpha_bil*alpha_msg * bracket(sum_{e->t} F[src_e], F[t])
    This removes the per-edge bracket entirely: only a scatter-add of raw
    source rows, then ONE bracket per node.
  * bracket(x, a*x) == 0 exactly (structure constants are antisymmetrized
    with zero diagonal), so the update bracket vanishes and
      out = F + agg.

Device mapping (8 NeuronCores, no collectives):
  Edges are bucketed host-side by target node; core c owns target nodes
  [2500c, 2500(c+1)) and all edges into them.  Per core:
    - dma_gather pulls the bf16 source rows (padded to 256 cols) from a
      replicated DRAM feature table; gathered edges land 1/partition.
    - per 128-node window, one-hot matmuls (edges on K) accumulate
      S^T = sum of source rows, feature-major, in PSUM (f32).
    - bracket via factorized matmuls: Gx = Q^T S^T, Gy = Q^T F^T,
      terms+/- = GxA*GyB / GxB*GyA (DVE), agg = terms^T @ P (cv folded in P).
    - out = F(f32) + agg, DMA'd out node-major.
"""

import numpy as np
import ml_dtypes

import concourse.bass as bass
import concourse.tile as tile
from concourse import bacc, mybir
from concourse.bass_utils import run_bass_kernel_spmd

BF16 = mybir.dt.bfloat16
F32 = mybir.dt.float32
I16 = mybir.dt.int16

N_NODES = 20000
D = 248
D_PAD = 256
N_CORES = 8
N_C = N_NODES // N_CORES          # 2500 target nodes per core
N_CPAD = 2560                     # padded to 20 windows of 128
N_WIN = N_CPAD // 128             # 20
NB = 300                          # base structure-constant triples
TS = 384                          # padded per-side t dim (3 chunks of 128)
NODE_CHUNK = 512                  # bracket node chunk (PSUM bank free dim)

_CACHE = {}

# debug bisection flags
ONEHOT_MODE = "batched"   # "batched" | "pergroup"
GATHER_MODE = "dma"       # "dma" | "memset"


def _build(g_w, chunk_windows):
    """Build + compile the SPMD program. g_w[w] = #128-edge groups for window
    w (uniform across cores); chunk_windows = list of lists of window ids per
    gather chunk."""
    tot_g = int(sum(g_w))
    g_off = np.concatenate([[0], np.cumsum(g_w)]).astype(int)  # group offset per window

    nc = bacc.Bacc("TRN2", target_bir_lowering=False, debug=False,
                   num_devices=N_CORES)

    ftable = nc.dram_tensor("ftable", [N_NODES + 1, D_PAD], BF16, kind="ExternalInput")
    gidx = nc.dram_tensor("gidx", [128, tot_g * 8], I16, kind="ExternalInput")
    tgtcols = nc.dram_tensor("tgtcols", [128, tot_g], BF16, kind="ExternalInput")
    iotac = nc.dram_tensor("iotac", [128, 128], BF16, kind="ExternalInput")
    qmat = nc.dram_tensor("qmat", [D_PAD, 2 * TS], BF16, kind="ExternalInput")
    pmat = nc.dram_tensor("pmat", [2 * TS, D_PAD], BF16, kind="ExternalInput")
    ftr = nc.dram_tensor("ftr", [D_PAD, N_CPAD], BF16, kind="ExternalInput")
    fnode = nc.dram_tensor("fnode", [N_CPAD, D_PAD], F32, kind="ExternalInput")
    out_d = nc.dram_tensor("out", [N_CPAD, D_PAD], F32, kind="ExternalOutput")

    n_chunks = len(chunk_windows)
    max_chunk_g = max(int(sum(g_w[w] for w in cw)) for cw in chunk_windows)

    with tile.TileContext(nc) as tc:
        with tc.tile_pool(name="const", bufs=1) as cpool, \
             tc.tile_pool(name="gpool", bufs=2) as gpool, \
             tc.tile_pool(name="hpool", bufs=2) as hpool, \
             tc.tile_pool(name="work", bufs=2) as wpool, \
             tc.tile_pool(name="psum", bufs=1, space="PSUM") as pp:

            # ---- constant loads ----
            idx_sb = cpool.tile([128, tot_g * 8], I16, tag="idx")
            nc.sync.dma_start(out=idx_sb[:], in_=gidx.ap())
            tcol_sb = cpool.tile([128, tot_g], BF16, tag="tcol")
            nc.sync.dma_start(out=tcol_sb[:], in_=tgtcols.ap())
            iota_sb = cpool.tile([128, 128], BF16, tag="iota")
            nc.sync.dma_start(out=iota_sb[:], in_=iotac.ap())
            q_sb = [cpool.tile([128, 2 * TS], BF16, tag=f"q{h}", name=f"q{h}") for h in range(2)]
            for h in range(2):
                nc.sync.dma_start(out=q_sb[h][:], in_=qmat.ap()[h * 128:(h + 1) * 128, :])
            p_sb = [cpool.tile([128, D_PAD], BF16, tag=f"p{m}", name=f"p{m}") for m in range(6)]
            for m in range(6):
                nc.sync.dma_start(out=p_sb[m][:], in_=pmat.ap()[m * 128:(m + 1) * 128, :])
            ftr_sb = [cpool.tile([128, N_CPAD], BF16, tag=f"ftr{h}", name=f"ftr{h}") for h in range(2)]
            for h in range(2):
                nc.sync.dma_start(out=ftr_sb[h][:], in_=ftr.ap()[h * 128:(h + 1) * 128, :])

            # ---- Gy = Q^T F^T  (per m-chunk per node-chunk) ----
            n_nchunks = N_CPAD // NODE_CHUNK  # 5
            gy_sb = [[None] * n_nchunks for _ in range(6)]
            for cn in range(n_nchunks):
                nsl = slice(cn * NODE_CHUNK, (cn + 1) * NODE_CHUNK)
                for m in range(6):
                    pt = pp.tile([128, NODE_CHUNK], F32, tag="gxy")
                    msl = slice(m * 128, (m + 1) * 128)
                    nc.tensor.matmul(out=pt[:], lhsT=q_sb[0][:, msl],
                                     rhs=ftr_sb[0][:, nsl], start=True, stop=False)
                    nc.tensor.matmul(out=pt[:], lhsT=q_sb[1][:, msl],
                                     rhs=ftr_sb[1][:, nsl], start=False, stop=True)
                    gt = wpool.tile([128, NODE_CHUNK], BF16, tag=f"gy{m}_{cn}")
                    nc.vector.tensor_copy(out=gt[:], in_=pt[:])
                    gy_sb[m][cn] = gt

            # ---- gather chunks + scatter one-hot matmuls ----
            sT = [cpool.tile([128, N_CPAD], BF16, tag=f"sT{h}", name=f"sT{h}") for h in range(2)]
            idx_pos = 0  # running index offset (in idx elements)
            for cw in chunk_windows:
                cg = int(sum(g_w[w] for w in cw))
                n_idx = cg * 128
                g_t = gpool.tile([128, max_chunk_g, D_PAD], BF16, tag="G")
                if GATHER_MODE == "dma":
                    nc.gpsimd.dma_gather(
                        out_ap=g_t[:, :cg, :],
                        in_ap=ftable.ap(),
                        idxs_ap=idx_sb[:, idx_pos // 16:(idx_pos + n_idx) // 16],
                        num_idxs=n_idx,
                        num_idxs_reg=n_idx,
                        elem_size=D_PAD,
                        single_packet=False,
                    )
                else:
                    nc.vector.memset(g_t[:, :cg, :], 0.25)
                c_goff = idx_pos // 128  # global group index of chunk start
                idx_pos += n_idx
                for w in cw:
                    gw = int(g_w[w])
                    # batched one-hot build for the whole window
                    h_t = hpool.tile([128, gw * 128], BF16, tag="H")
                    if ONEHOT_MODE == "batched":
                        in0 = bass.AP(iota_sb[:].tensor, iota_sb[:].offset,
                                      [[128, 128], [0, gw], [1, 128]])
                        tsl = tcol_sb[:, g_off[w]:g_off[w] + gw]
                        in1 = bass.AP(tsl.tensor, tsl.offset,
                                      [[tot_g, 128], [1, gw], [0, 128]])
                        outap = bass.AP(h_t[:].tensor, h_t[:].offset,
                                        [[gw * 128, 128], [128, gw], [1, 128]])
                        nc.vector.tensor_tensor(out=outap, in0=in0, in1=in1,
                                                op=mybir.AluOpType.is_equal)
                    else:
                        for g in range(gw):
                            nc.vector.tensor_scalar(
                                out=h_t[:, g * 128:(g + 1) * 128],
                                in0=iota_sb[:],
                                scalar1=tcol_sb[:, g_off[w] + g:g_off[w] + g + 1],
                                scalar2=None,
                                op0=mybir.AluOpType.is_equal,
                            )
                    ps = [pp.tile([128, 128], F32, tag="swin", name=f"ps{w}_{_h}") for _h in range(2)]
                    for g in range(gw):
                        slot = g_off[w] + g - c_goff
                        for h in range(2):
                            nc.tensor.matmul(
                                out=ps[h][:],
                                lhsT=g_t[:, slot, h * 128:(h + 1) * 128],
                                rhs=h_t[:, g * 128:(g + 1) * 128],
                                start=(g == 0), stop=(g == gw - 1),
                            )
                    for h in range(2):
                        nc.vector.tensor_copy(
                            out=sT[h][:, w * 128:(w + 1) * 128], in_=ps[h][:])

            # ---- bracket + output ----
            for cn in range(n_nchunks):
                nsl = slice(cn * NODE_CHUNK, (cn + 1) * NODE_CHUNK)
                terms = [None] * 6
                for m in range(6):
                    pt = pp.tile([128, NODE_CHUNK], F32, tag="gxy")
                    msl = slice(m * 128, (m + 1) * 128)
                    nc.tensor.matmul(out=pt[:], lhsT=q_sb[0][:, msl],
                                     rhs=sT[0][:, nsl], start=True, stop=False)
                    nc.tensor.matmul(out=pt[:], lhsT=q_sb[1][:, msl],
                                     rhs=sT[1][:, nsl], start=False, stop=True)
                    tm = wpool.tile([128, NODE_CHUNK], BF16, tag=f"terms{m}")
                    gy_other = gy_sb[m + 3][cn] if m < 3 else gy_sb[m - 3][cn]
                    nc.vector.tensor_tensor(out=tm[:], in0=pt[:], in1=gy_other[:],
                                            op=mybir.AluOpType.mult)
                    terms[m] = tm
                for nt in range(NODE_CHUNK // 128):
                    po = pp.tile([128, D_PAD], F32, tag="out")
                    for m in range(6):
                        nc.tensor.matmul(out=po[:],
                                         lhsT=terms[m][:, nt * 128:(nt + 1) * 128],
                                         rhs=p_sb[m][:],
                                         start=(m == 0), stop=(m == 5))
                    r0 = cn * NODE_CHUNK + nt * 128
                    fnt = wpool.tile([128, D_PAD], F32, tag="fn")
                    nc.sync.dma_start(out=fnt[:], in_=fnode.ap()[r0:r0 + 128, :])
                    osb = wpool.tile([128, D_PAD], F32, tag="osb")
                    nc.vector.tensor_tensor(out=osb[:], in0=po[:], in1=fnt[:],
                                            op=mybir.AluOpType.add)
                    nc.sync.dma_start(out=out_d.ap()[r0:r0 + 128, :], in_=osb[:])

    nc.compile()
    return nc


def _prep(features, edge_index, ci, cj, ck, cv,
          alpha_msg, alpha_bil, alpha_w, update_scale):
    F = np.asarray(features, np.float32)
    ei = np.asarray(edge_index)
    ci = np.asarray(ci); cj = np.asarray(cj); ck = np.asarray(ck)
    cv = np.asarray(cv, np.float32)
    am = float(alpha_msg); ab = float(alpha_bil)
    src, tgt = ei[0].astype(np.int64), ei[1].astype(np.int64)

    bf = ml_dtypes.bfloat16

    # bucket edges by (core, window)
    core = tgt // N_C
    tl = tgt - core * N_C            # target local id in [0, 2500)
    win = tl // 128
    # per (core, window) edge lists
    order = np.lexsort((tl, win, core))
    src_s, core_s, win_s, tl_s = src[order], core[order], win[order], tl[order]
    counts = np.zeros((N_CORES, N_WIN), np.int64)
    np.add.at(counts, (core_s, win_s), 1)
    g_w = np.maximum(1, np.ceil(counts.max(axis=0) / 128).astype(np.int64))
    tot_g = int(g_w.sum())
    g_offs = np.concatenate([[0], np.cumsum(g_w)]).astype(int)

    # gather chunks: split windows into ~5 chunks balanced by group count
    n_chunks = 5
    chunk_windows, cur, cur_g = [], [], 0
    target = tot_g / n_chunks
    for w in range(N_WIN):
        cur.append(w); cur_g += g_w[w]
        if cur_g >= target and len(chunk_windows) < n_chunks - 1:
            chunk_windows.append(cur); cur, cur_g = [], 0
    chunk_windows.append(cur)

    # per-core idx + tgtcol arrays
    tot_idx = tot_g * 128
    idx_all = np.zeros((N_CORES, tot_idx), np.int16)
    col_all = np.full((N_CORES, tot_idx), -1.0, np.float32)
    # start offset of each (core, window) run inside the sorted arrays
    run_starts = np.zeros((N_CORES, N_WIN), np.int64)
    np.cumsum(counts.ravel()[:-1], out=run_starts.ravel()[1:])
    for c in range(N_CORES):
        for w in range(N_WIN):
            cnt = int(counts[c, w]); s0 = int(run_starts[c, w])
            base = g_offs[w] * 128
            idx_all[c, base:base + cnt] = src_s[s0:s0 + cnt].astype(np.int16)
            col_all[c, base:base + cnt] = (tl_s[s0:s0 + cnt] - w * 128).astype(np.float32)

    in_maps = []
    ftable = np.zeros((N_NODES + 1, D_PAD), bf)
    ftable[:N_NODES, :D] = F.astype(bf)
    iota = np.broadcast_to(np.arange(128, dtype=np.float32), (128, 128)).astype(bf)

    # Q (D_PAD, 2*TS): A side cols [0,300) select ci; B side cols [TS, TS+300) select cj
    Q = np.zeros((D_PAD, 2 * TS), np.float32)
    i_s, j_s, k_s, v_s = ci[:NB], cj[:NB], ck[:NB], cv[:NB]
    Q[i_s, np.arange(NB)] = 1.0
    Q[j_s, TS + np.arange(NB)] = 1.0
    # P (2*TS, D_PAD): rows [0,300) -> +v at k ; rows [TS, TS+300) -> -v at k
    scale = ab * am
    P = np.zeros((2 * TS, D_PAD), np.float32)
    P[np.arange(NB), k_s] = v_s * scale
    P[TS + np.arange(NB), k_s] = -v_s * scale

    for c in range(N_CORES):
        wrapped = idx_all[c].reshape(tot_idx // 16, 16).T     # (16, tot_idx/16)
        gidx = np.tile(wrapped, (8, 1)).copy()                # (128, tot_idx/16)
        tcols = col_all[c].reshape(tot_g, 128).T.astype(bf).copy()  # (128, tot_g)
        fslice = np.zeros((N_CPAD, D_PAD), np.float32)
        fslice[:N_C, :D] = F[c * N_C:(c + 1) * N_C]
        ftr_c = np.zeros((D_PAD, N_CPAD), bf)
        ftr_c[:D, :N_C] = F[c * N_C:(c + 1) * N_C].T.astype(bf)
        in_maps.append({
            "ftable": ftable,
            "gidx": gidx,
            "tgtcols": tcols,
            "iotac": iota,
            "qmat": Q.astype(bf),
            "pmat": P.astype(bf),
            "ftr": ftr_c,
            "fnode": fslice,
        })
    return tuple(g_w.tolist()), tuple(tuple(cw) for cw in chunk_windows), in_maps


def _run(in_maps, nc, trace=False):
    res = run_bass_kernel_spmd(nc, in_maps, core_ids=list(range(N_CORES)),
                               trace=trace)
    out = np.empty((N_NODES, D), np.float32)
    for c in range(N_CORES):
        out[c * N_C:(c + 1) * N_C] = res.results[c]["out"][:N_C, :D]
    return out, res


def kernel(**inputs):
    g_w, chunk_windows, in_maps = _prep(**inputs)
    key = (g_w, chunk_windows)
    if key not in _CACHE:
        _CACHE[key] = _build(np.array(g_w), [list(cw) for cw in chunk_windows])
    out, _ = _run(in_maps, _CACHE[key], trace=False)
    return out


def kernel_traced(**inputs):
    """test-only entry: returns (out, BassKernelResults with exec_time_ns)."""
    g_w, chunk_windows, in_maps = _prep(**inputs)
    key = (g_w, chunk_windows)
    if key not in _CACHE:
        _CACHE[key] = _build(np.array(g_w), [list(cw) for cw in chunk_windows])
    return _run(in_maps, _CACHE[key], trace=True)


# revision 7
# speedup vs baseline: 1.1098x; 1.1098x over previous
"""Trainium2 Bass kernel for EquivariantLieConvLayer (GNN message passing).

Math restructuring (exact algebra, not approximation):
  reference computes, per edge e = (s -> t):
      msg_e = alpha_bil * bracket(alpha_msg * F[s], F[t])
      agg[t] += msg_e
      out = F + agg + update_scale * bracket(agg, alpha_w * agg)
  * bracket is bilinear and F[t] is shared by all edges targeting t, so
      agg[t] = alpha_bil*alpha_msg * bracket(sum_{e->t} F[src_e], F[t])
    This removes the per-edge bracket entirely: only a scatter-add of raw
    source rows, then ONE bracket per node.
  * bracket(x, a*x) == 0 exactly (structure constants are antisymmetrized
    with zero diagonal), so the update bracket vanishes and
      out = F + agg.

Device mapping (8 NeuronCores, no collectives):
  Target nodes are assigned host-side to 160 (core, window) bins of <=128
  nodes, balancing per-bin in-edge counts so every bin needs the same
  number of 128-edge groups (SPMD-uniform instruction stream).  Per core:
    - dma_gather pulls bf16 source rows (padded to 256 cols) from a
      replicated DRAM feature table; gathered edges land 1/partition.
    - per window, one-hot matmuls (edges on K) accumulate
      S^T = sum of source rows, feature-major, in PSUM (f32).
    - bracket via factorized matmuls: Gx = Q^T S^T, Gy = Q^T F^T,
      terms+/- = GxA*GyB / GxB*GyA (DVE), agg = terms^T @ P (cv folded in P).
    - out = F(f32) + agg, DMA'd out node-major; host unpermutes rows.
"""

import numpy as np
import ml_dtypes

import concourse.bass as bass
import concourse.tile as tile
from concourse import bacc, mybir
from concourse.bass_utils import run_bass_kernel_spmd

BF16 = mybir.dt.bfloat16
F32 = mybir.dt.float32
I16 = mybir.dt.int16

N_NODES = 20000
D = 248
D_PAD = 256
N_CORES = 8
N_CPAD = 2560                     # padded node slots per core: 20 windows of 128
N_WIN = N_CPAD // 128             # 20
NB = 300                          # base structure-constant triples
TS = 384                          # padded per-side t dim (3 chunks of 128)
NODE_CHUNK = 512                  # bracket node chunk (PSUM bank free dim)

_CACHE = {}


def _build(g_w, chunk_windows):
    """Build + compile the SPMD program. g_w[w] = #128-edge groups for window
    w (uniform across cores); chunk_windows = list of window-id lists per
    gather chunk."""
    tot_g = int(sum(g_w))
    g_off = np.concatenate([[0], np.cumsum(g_w)]).astype(int)

    nc = bacc.Bacc("TRN2", target_bir_lowering=False, debug=False,
                   num_devices=N_CORES)

    ftable = nc.dram_tensor("ftable", [N_NODES + 1, D_PAD], BF16, kind="ExternalInput")
    gidx = nc.dram_tensor("gidx", [128, tot_g * 8], I16, kind="ExternalInput")
    tgtcols = nc.dram_tensor("tgtcols", [128, tot_g], BF16, kind="ExternalInput")
    iotac = nc.dram_tensor("iotac", [128, 128], BF16, kind="ExternalInput")
    qmat = nc.dram_tensor("qmat", [D_PAD, 2 * TS], BF16, kind="ExternalInput")
    pmat = nc.dram_tensor("pmat", [2 * TS, D_PAD], BF16, kind="ExternalInput")
    ftr = nc.dram_tensor("ftr", [D_PAD, N_CPAD], BF16, kind="ExternalInput")
    fnode = nc.dram_tensor("fnode", [N_CPAD, D_PAD], F32, kind="ExternalInput")
    out_d = nc.dram_tensor("out", [N_CPAD, D_PAD], F32, kind="ExternalOutput")

    max_chunk_g = max(int(sum(g_w[w] for w in cw)) for cw in chunk_windows)
    n_nchunks = N_CPAD // NODE_CHUNK  # 5

    with tile.TileContext(nc) as tc:
        with tc.tile_pool(name="const", bufs=1) as cpool, \
             tc.tile_pool(name="gpool", bufs=3) as gpool, \
             tc.tile_pool(name="hpool", bufs=2) as hpool, \
             tc.tile_pool(name="work", bufs=2) as wpool, \
             tc.tile_pool(name="psum", bufs=1, space="PSUM") as pp:

            # ---- idx load first, then gathers ASAP (Pool engine = critical path)
            idx_sb = cpool.tile([128, tot_g * 8], I16, tag="idx")
            nc.sync.dma_start(out=idx_sb[:], in_=gidx.ap())

            g_tiles = []
            idx_pos = 0
            for cw in chunk_windows:
                cg = int(sum(g_w[w] for w in cw))
                n_idx = cg * 128
                g_t = gpool.tile([128, max_chunk_g, D_PAD], BF16, tag="G",
                                 name=f"G{len(g_tiles)}")
                nc.gpsimd.dma_gather(
                    out_ap=g_t[:, :cg, :],
                    in_ap=ftable.ap(),
                    idxs_ap=idx_sb[:, idx_pos // 16:(idx_pos + n_idx) // 16],
                    num_idxs=n_idx,
                    num_idxs_reg=n_idx,
                    elem_size=D_PAD,
                    single_packet=False,
                )
                g_tiles.append((g_t, idx_pos // 128))
                idx_pos += n_idx

            # ---- remaining constant loads ----
            tcol_sb = cpool.tile([128, tot_g], BF16, tag="tcol")
            nc.sync.dma_start(out=tcol_sb[:], in_=tgtcols.ap())
            iota_sb = cpool.tile([128, 128], BF16, tag="iota")
            nc.sync.dma_start(out=iota_sb[:], in_=iotac.ap())
            q_sb = [cpool.tile([128, 2 * TS], BF16, tag=f"q{h}", name=f"q{h}")
                    for h in range(2)]
            for h in range(2):
                nc.sync.dma_start(out=q_sb[h][:], in_=qmat.ap()[h * 128:(h + 1) * 128, :])
            p_sb = [cpool.tile([128, D_PAD], BF16, tag=f"p{m}", name=f"p{m}")
                    for m in range(6)]
            for m in range(6):
                nc.sync.dma_start(out=p_sb[m][:], in_=pmat.ap()[m * 128:(m + 1) * 128, :])
            ftr_sb = [cpool.tile([128, N_CPAD], BF16, tag=f"ftr{h}", name=f"ftr{h}")
                      for h in range(2)]
            for h in range(2):
                nc.sync.dma_start(out=ftr_sb[h][:], in_=ftr.ap()[h * 128:(h + 1) * 128, :])

            # ---- Gy = Q^T F^T (PE filler while gathers generate) ----
            gy_sb = [[None] * n_nchunks for _ in range(6)]
            for cn in range(n_nchunks):
                nsl = slice(cn * NODE_CHUNK, (cn + 1) * NODE_CHUNK)
                for m in range(6):
                    pt = pp.tile([128, NODE_CHUNK], F32, tag="gxy", bufs=2,
                                 name=f"gyp{cn}_{m}")
                    msl = slice(m * 128, (m + 1) * 128)
                    nc.tensor.matmul(out=pt[:], lhsT=q_sb[0][:, msl],
                                     rhs=ftr_sb[0][:, nsl], start=True, stop=False)
                    nc.tensor.matmul(out=pt[:], lhsT=q_sb[1][:, msl],
                                     rhs=ftr_sb[1][:, nsl], start=False, stop=True)
                    gt = wpool.tile([128, NODE_CHUNK], BF16, tag=f"gy{m}_{cn}",
                                    bufs=1, name=f"gy{m}_{cn}")
                    nc.vector.tensor_copy(out=gt[:], in_=pt[:])
                    gy_sb[m][cn] = gt

            # ---- scatter: per window, batched one-hot + accumulate S^T ----
            sT = [cpool.tile([128, N_CPAD], BF16, tag=f"sT{h}", name=f"sT{h}")
                  for h in range(2)]
            win_chunk = {}
            for ci, cw in enumerate(chunk_windows):
                for w in cw:
                    win_chunk[w] = ci
            for w in range(N_WIN):
                gw = int(g_w[w])
                g_t, c_goff = g_tiles[win_chunk[w]]
                h_t = hpool.tile([128, gw * 128], BF16, tag="H", name=f"H{w}")
                in0 = bass.AP(iota_sb[:].tensor, iota_sb[:].offset,
                              [[128, 128], [0, gw], [1, 128]])
                tsl = tcol_sb[:, g_off[w]:g_off[w] + gw]
                in1 = bass.AP(tsl.tensor, tsl.offset,
                              [[tot_g, 128], [1, gw], [0, 128]])
                outap = bass.AP(h_t[:].tensor, h_t[:].offset,
                                [[gw * 128, 128], [128, gw], [1, 128]])
                nc.vector.tensor_tensor(out=outap, in0=in0, in1=in1,
                                        op=mybir.AluOpType.is_equal)
                ps = [pp.tile([128, 128], F32, tag="swin", bufs=4,
                              name=f"ps{w}_{hh}") for hh in range(2)]
                for g in range(gw):
                    slot = g_off[w] + g - c_goff
                    for h in range(2):
                        nc.tensor.matmul(
                            out=ps[h][:],
                            lhsT=g_t[:, slot, h * 128:(h + 1) * 128],
                            rhs=h_t[:, g * 128:(g + 1) * 128],
                            start=(g == 0), stop=(g == gw - 1),
                        )
                for h in range(2):
                    nc.vector.tensor_copy(
                        out=sT[h][:, w * 128:(w + 1) * 128], in_=ps[h][:])

            # ---- bracket + output per node chunk ----
            for cn in range(n_nchunks):
                nsl = slice(cn * NODE_CHUNK, (cn + 1) * NODE_CHUNK)
                terms = [None] * 6
                for m in range(6):
                    pt = pp.tile([128, NODE_CHUNK], F32, tag="gxy", bufs=2,
                                 name=f"gxp{cn}_{m}")
                    msl = slice(m * 128, (m + 1) * 128)
                    nc.tensor.matmul(out=pt[:], lhsT=q_sb[0][:, msl],
                                     rhs=sT[0][:, nsl], start=True, stop=False)
                    nc.tensor.matmul(out=pt[:], lhsT=q_sb[1][:, msl],
                                     rhs=sT[1][:, nsl], start=False, stop=True)
                    tm = wpool.tile([128, NODE_CHUNK], BF16, tag=f"terms{m}",
                                    bufs=2, name=f"terms{m}_{cn}")
                    gy_other = gy_sb[m + 3][cn] if m < 3 else gy_sb[m - 3][cn]
                    nc.vector.tensor_tensor(out=tm[:], in0=pt[:], in1=gy_other[:],
                                            op=mybir.AluOpType.mult)
                    terms[m] = tm
                for nt in range(NODE_CHUNK // 128):
                    po = pp.tile([128, D_PAD], F32, tag="out", bufs=2,
                                 name=f"po{cn}_{nt}")
                    for m in range(6):
                        nc.tensor.matmul(out=po[:],
                                         lhsT=terms[m][:, nt * 128:(nt + 1) * 128],
                                         rhs=p_sb[m][:],
                                         start=(m == 0), stop=(m == 5))
                    r0 = cn * NODE_CHUNK + nt * 128
                    fnt = wpool.tile([128, D_PAD], F32, tag="fn", bufs=3,
                                     name=f"fn{cn}_{nt}")
                    nc.sync.dma_start(out=fnt[:], in_=fnode.ap()[r0:r0 + 128, :])
                    osb = wpool.tile([128, D_PAD], F32, tag="osb", bufs=3,
                                     name=f"osb{cn}_{nt}")
                    nc.vector.tensor_tensor(out=osb[:], in0=po[:], in1=fnt[:],
                                            op=mybir.AluOpType.add)
                    nc.sync.dma_start(out=out_d.ap()[r0:r0 + 128, :], in_=osb[:])

    nc.compile()
    return nc


def _prep(features, edge_index, ci, cj, ck, cv,
          alpha_msg, alpha_bil, alpha_w, update_scale):
    F = np.asarray(features, np.float32)
    ei = np.asarray(edge_index)
    ci = np.asarray(ci); cj = np.asarray(cj); ck = np.asarray(ck)
    cv = np.asarray(cv, np.float32)
    am = float(alpha_msg); ab = float(alpha_bil)
    src, tgt = ei[0].astype(np.int64), ei[1].astype(np.int64)
    bf = ml_dtypes.bfloat16
    n_bins = N_CORES * N_WIN

    # --- balanced assignment of nodes to (core, window) bins ---
    deg = np.bincount(tgt, minlength=N_NODES)
    order = np.argsort(-deg, kind="stable")
    bin_load = np.zeros(n_bins, np.int64)
    bin_fill = np.zeros(n_bins, np.int64)
    node_bin = np.empty(N_NODES, np.int64)
    node_slot = np.empty(N_NODES, np.int64)
    import heapq
    heap = [(0, b) for b in range(n_bins)]
    heapq.heapify(heap)
    for n in order:
        while True:
            load, b = heapq.heappop(heap)
            if bin_fill[b] < 128:
                break
        node_bin[n] = b
        node_slot[n] = bin_fill[b]
        bin_fill[b] += 1
        bin_load[b] = load + deg[n]
        if bin_fill[b] < 128:
            heapq.heappush(heap, (bin_load[b], b))

    g_w_all = np.ceil(bin_load.reshape(N_CORES, N_WIN) / 128).astype(np.int64)
    g_w = np.maximum(1, g_w_all.max(axis=0))
    tot_g = int(g_w.sum())
    g_offs = np.concatenate([[0], np.cumsum(g_w)]).astype(int)

    # local (padded) node id within a core for each node
    node_core = node_bin // N_WIN
    node_win = node_bin % N_WIN
    node_local = node_win * 128 + node_slot          # in [0, 2560)

    # gather chunks: split windows into ~5 chunks balanced by group count
    n_chunks = 5
    chunk_windows, cur, cur_g = [], [], 0
    target = tot_g / n_chunks
    for w in range(N_WIN):
        cur.append(w); cur_g += g_w[w]
        if cur_g >= target and len(chunk_windows) < n_chunks - 1:
            chunk_windows.append(cur); cur, cur_g = [], 0
    chunk_windows.append(cur)

    # --- per-core edge slots ---
    e_core = node_core[tgt]
    e_win = node_win[tgt]
    tot_idx = tot_g * 128
    idx_all = np.zeros((N_CORES, tot_idx), np.int16)
    col_all = np.full((N_CORES, tot_idx), -1.0, np.float32)
    eorder = np.lexsort((tgt, e_win, e_core))
    src_s = src[eorder]; core_s = e_core[eorder]; win_s = e_win[eorder]
    tl_s = (node_local[tgt] - node_win[tgt] * 128)[eorder]  # slot within window
    counts = np.zeros((N_CORES, N_WIN), np.int64)
    np.add.at(counts, (core_s, win_s), 1)
    run_starts = np.zeros((N_CORES, N_WIN), np.int64)
    np.cumsum(counts.ravel()[:-1], out=run_starts.ravel()[1:])
    for c in range(N_CORES):
        for w in range(N_WIN):
            cnt = int(counts[c, w]); s0 = int(run_starts[c, w])
            base = g_offs[w] * 128
            idx_all[c, base:base + cnt] = src_s[s0:s0 + cnt].astype(np.int16)
            col_all[c, base:base + cnt] = tl_s[s0:s0 + cnt].astype(np.float32)

    # --- constant tables ---
    ftable = np.zeros((N_NODES + 1, D_PAD), bf)
    ftable[:N_NODES, :D] = F.astype(bf)
    iota = np.broadcast_to(np.arange(128, dtype=np.float32), (128, 128)).astype(bf)
    Q = np.zeros((D_PAD, 2 * TS), np.float32)
    i_s, j_s, k_s, v_s = ci[:NB], cj[:NB], ck[:NB], cv[:NB]
    Q[i_s, np.arange(NB)] = 1.0
    Q[j_s, TS + np.arange(NB)] = 1.0
    scale = ab * am
    P = np.zeros((2 * TS, D_PAD), np.float32)
    P[np.arange(NB), k_s] = v_s * scale
    P[TS + np.arange(NB), k_s] = -v_s * scale

    # permuted F slices per core
    in_maps = []
    # inverse map: (core, local) -> original node (or -1)
    inv = np.full((N_CORES, N_CPAD), -1, np.int64)
    inv[node_core, node_local] = np.arange(N_NODES)
    for c in range(N_CORES):
        wrapped = idx_all[c].reshape(tot_idx // 16, 16).T
        gidx = np.tile(wrapped, (8, 1)).copy()
        tcols = col_all[c].reshape(tot_g, 128).T.astype(bf).copy()
        sel = inv[c]
        valid = sel >= 0
        fslice = np.zeros((N_CPAD, D_PAD), np.float32)
        fslice[valid, :D] = F[sel[valid]]
        ftr_c = np.zeros((D_PAD, N_CPAD), bf)
        ftr_c[:D, valid] = F[sel[valid]].T.astype(bf)
        in_maps.append({
            "ftable": ftable,
            "gidx": gidx,
            "tgtcols": tcols,
            "iotac": iota,
            "qmat": Q.astype(bf),
            "pmat": P.astype(bf),
            "ftr": ftr_c,
            "fnode": fslice,
        })
    return (tuple(g_w.tolist()), tuple(tuple(cw) for cw in chunk_windows),
            in_maps, inv)


def _run(in_maps, inv, nc, trace=False):
    res = run_bass_kernel_spmd(nc, in_maps, core_ids=list(range(N_CORES)),
                               trace=trace)
    out = np.empty((N_NODES, D), np.float32)
    for c in range(N_CORES):
        sel = inv[c]
        valid = sel >= 0
        out[sel[valid]] = res.results[c]["out"][valid, :D]
    return out, res


def _get(inputs):
    g_w, chunk_windows, in_maps, inv = _prep(**inputs)
    key = (g_w, chunk_windows)
    if key not in _CACHE:
        _CACHE[key] = _build(np.array(g_w), [list(cw) for cw in chunk_windows])
    return in_maps, inv, _CACHE[key]


def kernel(**inputs):
    in_maps, inv, nc = _get(inputs)
    out, _ = _run(in_maps, inv, nc, trace=False)
    return out


def kernel_traced(**inputs):
    in_maps, inv, nc = _get(inputs)
    return _run(in_maps, inv, nc, trace=True)


# revision 8
# speedup vs baseline: 1.1158x; 1.0054x over previous
"""Trainium2 Bass kernel for EquivariantLieConvLayer (GNN message passing).

Math restructuring (exact algebra, not approximation):
  reference computes, per edge e = (s -> t):
      msg_e = alpha_bil * bracket(alpha_msg * F[s], F[t])
      agg[t] += msg_e
      out = F + agg + update_scale * bracket(agg, alpha_w * agg)
  * bracket is bilinear and F[t] is shared by all edges targeting t, so
      agg[t] = alpha_bil*alpha_msg * bracket(sum_{e->t} F[src_e], F[t])
    This removes the per-edge bracket entirely: only a scatter-add of raw
    source rows, then ONE bracket per node.
  * bracket(x, a*x) == 0 exactly (structure constants are antisymmetrized
    with zero diagonal), so the update bracket vanishes and
      out = F + agg.

Device mapping (8 NeuronCores, no collectives):
  Target nodes are assigned host-side to 160 (core, window) bins of <=128
  nodes, balancing per-bin in-edge counts so every bin needs the same
  number of 128-edge groups (SPMD-uniform instruction stream).  Per core:
    - dma_gather pulls bf16 source rows (padded to 256 cols) from a
      replicated DRAM feature table; gathered edges land 1/partition.
    - per window, one-hot matmuls (edges on K) accumulate
      S^T = sum of source rows, feature-major, in PSUM (f32).
    - bracket via factorized matmuls: Gx = Q^T S^T, Gy = Q^T F^T,
      terms+/- = GxA*GyB / GxB*GyA (DVE), agg = terms^T @ P (cv folded in P).
    - out = F(f32) + agg, DMA'd out node-major; host unpermutes rows.
"""

import numpy as np
import ml_dtypes

import concourse.bass as bass
import concourse.tile as tile
from concourse import bacc, mybir
from concourse.bass_utils import run_bass_kernel_spmd

BF16 = mybir.dt.bfloat16
F32 = mybir.dt.float32
I16 = mybir.dt.int16

N_NODES = 20000
D = 248
D_PAD = 256
N_CORES = 8
N_CPAD = 2560                     # padded node slots per core: 20 windows of 128
N_WIN = N_CPAD // 128             # 20
NB = 300                          # base structure-constant triples
TS = 384                          # padded per-side t dim (3 chunks of 128)
NODE_CHUNK = 256                  # bracket node chunk (2 windows)

_CACHE = {}


def _build(g_w, chunk_windows):
    """Build + compile the SPMD program. g_w[w] = #128-edge groups for window
    w (uniform across cores); chunk_windows = list of window-id lists per
    gather chunk."""
    tot_g = int(sum(g_w))
    g_off = np.concatenate([[0], np.cumsum(g_w)]).astype(int)

    nc = bacc.Bacc("TRN2", target_bir_lowering=False, debug=False,
                   num_devices=N_CORES)

    ftable = nc.dram_tensor("ftable", [N_NODES + 1, D_PAD], BF16, kind="ExternalInput")
    gidx = nc.dram_tensor("gidx", [128, tot_g * 8], I16, kind="ExternalInput")
    tgtcols = nc.dram_tensor("tgtcols", [128, tot_g], BF16, kind="ExternalInput")
    iotac = nc.dram_tensor("iotac", [128, 128], BF16, kind="ExternalInput")
    qmat = nc.dram_tensor("qmat", [D_PAD, 2 * TS], BF16, kind="ExternalInput")
    pmat = nc.dram_tensor("pmat", [2 * TS, D_PAD], BF16, kind="ExternalInput")
    ftr = nc.dram_tensor("ftr", [D_PAD, N_CPAD], BF16, kind="ExternalInput")
    fnode = nc.dram_tensor("fnode", [N_CPAD, D_PAD], F32, kind="ExternalInput")
    out_d = nc.dram_tensor("out", [N_CPAD, D_PAD], F32, kind="ExternalOutput")

    max_chunk_g = max(int(sum(g_w[w] for w in cw)) for cw in chunk_windows)
    n_nchunks = N_CPAD // NODE_CHUNK  # 5

    with tile.TileContext(nc) as tc:
        with tc.tile_pool(name="const", bufs=1) as cpool, \
             tc.tile_pool(name="gpool", bufs=3) as gpool, \
             tc.tile_pool(name="hpool", bufs=2) as hpool, \
             tc.tile_pool(name="work", bufs=2) as wpool, \
             tc.tile_pool(name="psum", bufs=1, space="PSUM") as pp:

            # ---- idx load first, then gathers ASAP (Pool engine = critical path)
            idx_sb = cpool.tile([128, tot_g * 8], I16, tag="idx")
            nc.sync.dma_start(out=idx_sb[:], in_=gidx.ap())

            g_tiles = []
            idx_pos = 0
            for cw in chunk_windows:
                cg = int(sum(g_w[w] for w in cw))
                n_idx = cg * 128
                g_t = gpool.tile([128, max_chunk_g, D_PAD], BF16, tag="G",
                                 name=f"G{len(g_tiles)}")
                nc.gpsimd.dma_gather(
                    out_ap=g_t[:, :cg, :],
                    in_ap=ftable.ap(),
                    idxs_ap=idx_sb[:, idx_pos // 16:(idx_pos + n_idx) // 16],
                    num_idxs=n_idx,
                    num_idxs_reg=n_idx,
                    elem_size=D_PAD,
                    single_packet=False,
                )
                g_tiles.append((g_t, idx_pos // 128))
                idx_pos += n_idx

            # ---- remaining constant loads ----
            tcol_sb = cpool.tile([128, tot_g], BF16, tag="tcol")
            nc.sync.dma_start(out=tcol_sb[:], in_=tgtcols.ap())
            iota_sb = cpool.tile([128, 128], BF16, tag="iota")
            nc.sync.dma_start(out=iota_sb[:], in_=iotac.ap())
            q_sb = [cpool.tile([128, 2 * TS], BF16, tag=f"q{h}", name=f"q{h}")
                    for h in range(2)]
            for h in range(2):
                nc.sync.dma_start(out=q_sb[h][:], in_=qmat.ap()[h * 128:(h + 1) * 128, :])
            p_sb = [cpool.tile([128, D_PAD], BF16, tag=f"p{m}", name=f"p{m}")
                    for m in range(6)]
            for m in range(6):
                nc.sync.dma_start(out=p_sb[m][:], in_=pmat.ap()[m * 128:(m + 1) * 128, :])
            ftr_sb = [cpool.tile([128, N_CPAD], BF16, tag=f"ftr{h}", name=f"ftr{h}")
                      for h in range(2)]
            for h in range(2):
                nc.sync.dma_start(out=ftr_sb[h][:], in_=ftr.ap()[h * 128:(h + 1) * 128, :])

            # ---- Gy = Q^T F^T (PE filler while gathers generate) ----
            gy_sb = [[None] * n_nchunks for _ in range(6)]
            for cn in range(n_nchunks):
                nsl = slice(cn * NODE_CHUNK, (cn + 1) * NODE_CHUNK)
                for m in range(6):
                    pt = pp.tile([128, NODE_CHUNK], F32, tag="gxy", bufs=2,
                                 name=f"gyp{cn}_{m}")
                    msl = slice(m * 128, (m + 1) * 128)
                    nc.tensor.matmul(out=pt[:], lhsT=q_sb[0][:, msl],
                                     rhs=ftr_sb[0][:, nsl], start=True, stop=False)
                    nc.tensor.matmul(out=pt[:], lhsT=q_sb[1][:, msl],
                                     rhs=ftr_sb[1][:, nsl], start=False, stop=True)
                    gt = wpool.tile([128, NODE_CHUNK], BF16, tag=f"gy{m}_{cn}",
                                    bufs=1, name=f"gy{m}_{cn}")
                    nc.vector.tensor_copy(out=gt[:], in_=pt[:])
                    gy_sb[m][cn] = gt

            # ---- scatter: per window, batched one-hot + accumulate S^T ----
            sT = [cpool.tile([128, N_CPAD], BF16, tag=f"sT{h}", name=f"sT{h}")
                  for h in range(2)]
            win_chunk = {}
            for ci, cw in enumerate(chunk_windows):
                for w in cw:
                    win_chunk[w] = ci
            for w in range(N_WIN):
                gw = int(g_w[w])
                g_t, c_goff = g_tiles[win_chunk[w]]
                h_t = hpool.tile([128, gw * 128], BF16, tag="H", name=f"H{w}")
                in0 = bass.AP(iota_sb[:].tensor, iota_sb[:].offset,
                              [[128, 128], [0, gw], [1, 128]])
                tsl = tcol_sb[:, g_off[w]:g_off[w] + gw]
                in1 = bass.AP(tsl.tensor, tsl.offset,
                              [[tot_g, 128], [1, gw], [0, 128]])
                outap = bass.AP(h_t[:].tensor, h_t[:].offset,
                                [[gw * 128, 128], [128, gw], [1, 128]])
                nc.vector.tensor_tensor(out=outap, in0=in0, in1=in1,
                                        op=mybir.AluOpType.is_equal)
                ps = [pp.tile([128, 128], F32, tag="swin", bufs=4,
                              name=f"ps{w}_{hh}") for hh in range(2)]
                for g in range(gw):
                    slot = g_off[w] + g - c_goff
                    for h in range(2):
                        nc.tensor.matmul(
                            out=ps[h][:],
                            lhsT=g_t[:, slot, h * 128:(h + 1) * 128],
                            rhs=h_t[:, g * 128:(g + 1) * 128],
                            start=(g == 0), stop=(g == gw - 1),
                        )
                for h in range(2):
                    nc.vector.tensor_copy(
                        out=sT[h][:, w * 128:(w + 1) * 128], in_=ps[h][:])

            # ---- bracket + output per node chunk ----
            for cn in range(n_nchunks):
                nsl = slice(cn * NODE_CHUNK, (cn + 1) * NODE_CHUNK)
                terms = [None] * 6
                for m in range(6):
                    pt = pp.tile([128, NODE_CHUNK], F32, tag="gxy", bufs=2,
                                 name=f"gxp{cn}_{m}")
                    msl = slice(m * 128, (m + 1) * 128)
                    nc.tensor.matmul(out=pt[:], lhsT=q_sb[0][:, msl],
                                     rhs=sT[0][:, nsl], start=True, stop=False)
                    nc.tensor.matmul(out=pt[:], lhsT=q_sb[1][:, msl],
                                     rhs=sT[1][:, nsl], start=False, stop=True)
                    tm = wpool.tile([128, NODE_CHUNK], BF16, tag=f"terms{m}",
                                    bufs=2, name=f"terms{m}_{cn}")
                    gy_other = gy_sb[m + 3][cn] if m < 3 else gy_sb[m - 3][cn]
                    nc.vector.tensor_tensor(out=tm[:], in0=pt[:], in1=gy_other[:],
                                            op=mybir.AluOpType.mult)
                    terms[m] = tm
                for nt in range(NODE_CHUNK // 128):
                    po = pp.tile([128, D_PAD], F32, tag="out", bufs=2,
                                 name=f"po{cn}_{nt}")
                    for m in range(6):
                        nc.tensor.matmul(out=po[:],
                                         lhsT=terms[m][:, nt * 128:(nt + 1) * 128],
                                         rhs=p_sb[m][:],
                                         start=(m == 0), stop=(m == 5))
                    r0 = cn * NODE_CHUNK + nt * 128
                    fnt = wpool.tile([128, D_PAD], F32, tag="fn", bufs=3,
                                     name=f"fn{cn}_{nt}")
                    nc.sync.dma_start(out=fnt[:], in_=fnode.ap()[r0:r0 + 128, :])
                    osb = wpool.tile([128, D_PAD], F32, tag="osb", bufs=3,
                                     name=f"osb{cn}_{nt}")
                    nc.vector.tensor_tensor(out=osb[:], in0=po[:], in1=fnt[:],
                                            op=mybir.AluOpType.add)
                    nc.sync.dma_start(out=out_d.ap()[r0:r0 + 128, :], in_=osb[:])

    nc.compile()
    return nc


def _prep(features, edge_index, ci, cj, ck, cv,
          alpha_msg, alpha_bil, alpha_w, update_scale):
    F = np.asarray(features, np.float32)
    ei = np.asarray(edge_index)
    ci = np.asarray(ci); cj = np.asarray(cj); ck = np.asarray(ck)
    cv = np.asarray(cv, np.float32)
    am = float(alpha_msg); ab = float(alpha_bil)
    src, tgt = ei[0].astype(np.int64), ei[1].astype(np.int64)
    bf = ml_dtypes.bfloat16
    n_bins = N_CORES * N_WIN

    # --- balanced assignment of nodes to (core, window) bins ---
    deg = np.bincount(tgt, minlength=N_NODES)
    order = np.argsort(-deg, kind="stable")
    bin_load = np.zeros(n_bins, np.int64)
    bin_fill = np.zeros(n_bins, np.int64)
    node_bin = np.empty(N_NODES, np.int64)
    node_slot = np.empty(N_NODES, np.int64)
    import heapq
    heap = [(0, b) for b in range(n_bins)]
    heapq.heapify(heap)
    for n in order:
        while True:
            load, b = heapq.heappop(heap)
            if bin_fill[b] < 128:
                break
        node_bin[n] = b
        node_slot[n] = bin_fill[b]
        bin_fill[b] += 1
        bin_load[b] = load + deg[n]
        if bin_fill[b] < 128:
            heapq.heappush(heap, (bin_load[b], b))

    g_w_all = np.ceil(bin_load.reshape(N_CORES, N_WIN) / 128).astype(np.int64)
    g_w = np.maximum(1, g_w_all.max(axis=0))
    tot_g = int(g_w.sum())
    g_offs = np.concatenate([[0], np.cumsum(g_w)]).astype(int)

    # local (padded) node id within a core for each node
    node_core = node_bin // N_WIN
    node_win = node_bin % N_WIN
    node_local = node_win * 128 + node_slot          # in [0, 2560)

    # gather chunks: 2 windows per chunk, last two windows solo (short tail)
    chunk_windows = [[w, w + 1] for w in range(0, N_WIN - 2, 2)]
    chunk_windows += [[N_WIN - 2], [N_WIN - 1]]

    # --- per-core edge slots ---
    e_core = node_core[tgt]
    e_win = node_win[tgt]
    tot_idx = tot_g * 128
    idx_all = np.zeros((N_CORES, tot_idx), np.int16)
    col_all = np.full((N_CORES, tot_idx), -1.0, np.float32)
    eorder = np.lexsort((tgt, e_win, e_core))
    src_s = src[eorder]; core_s = e_core[eorder]; win_s = e_win[eorder]
    tl_s = (node_local[tgt] - node_win[tgt] * 128)[eorder]  # slot within window
    counts = np.zeros((N_CORES, N_WIN), np.int64)
    np.add.at(counts, (core_s, win_s), 1)
    run_starts = np.zeros((N_CORES, N_WIN), np.int64)
    np.cumsum(counts.ravel()[:-1], out=run_starts.ravel()[1:])
    for c in range(N_CORES):
        for w in range(N_WIN):
            cnt = int(counts[c, w]); s0 = int(run_starts[c, w])
            base = g_offs[w] * 128
            idx_all[c, base:base + cnt] = src_s[s0:s0 + cnt].astype(np.int16)
            col_all[c, base:base + cnt] = tl_s[s0:s0 + cnt].astype(np.float32)

    # --- constant tables ---
    ftable = np.zeros((N_NODES + 1, D_PAD), bf)
    ftable[:N_NODES, :D] = F.astype(bf)
    iota = np.broadcast_to(np.arange(128, dtype=np.float32), (128, 128)).astype(bf)
    Q = np.zeros((D_PAD, 2 * TS), np.float32)
    i_s, j_s, k_s, v_s = ci[:NB], cj[:NB], ck[:NB], cv[:NB]
    Q[i_s, np.arange(NB)] = 1.0
    Q[j_s, TS + np.arange(NB)] = 1.0
    scale = ab * am
    P = np.zeros((2 * TS, D_PAD), np.float32)
    P[np.arange(NB), k_s] = v_s * scale
    P[TS + np.arange(NB), k_s] = -v_s * scale

    # permuted F slices per core
    in_maps = []
    # inverse map: (core, local) -> original node (or -1)
    inv = np.full((N_CORES, N_CPAD), -1, np.int64)
    inv[node_core, node_local] = np.arange(N_NODES)
    for c in range(N_CORES):
        wrapped = idx_all[c].reshape(tot_idx // 16, 16).T
        gidx = np.tile(wrapped, (8, 1)).copy()
        tcols = col_all[c].reshape(tot_g, 128).T.astype(bf).copy()
        sel = inv[c]
        valid = sel >= 0
        fslice = np.zeros((N_CPAD, D_PAD), np.float32)
        fslice[valid, :D] = F[sel[valid]]
        ftr_c = np.zeros((D_PAD, N_CPAD), bf)
        ftr_c[:D, valid] = F[sel[valid]].T.astype(bf)
        in_maps.append({
            "ftable": ftable,
            "gidx": gidx,
            "tgtcols": tcols,
            "iotac": iota,
            "qmat": Q.astype(bf),
            "pmat": P.astype(bf),
            "ftr": ftr_c,
            "fnode": fslice,
        })
    return (tuple(g_w.tolist()), tuple(tuple(cw) for cw in chunk_windows),
            in_maps, inv)


def _run(in_maps, inv, nc, trace=False):
    res = run_bass_kernel_spmd(nc, in_maps, core_ids=list(range(N_CORES)),
                               trace=trace)
    out = np.empty((N_NODES, D), np.float32)
    for c in range(N_CORES):
        sel = inv[c]
        valid = sel >= 0
        out[sel[valid]] = res.results[c]["out"][valid, :D]
    return out, res


def _get(inputs):
    g_w, chunk_windows, in_maps, inv = _prep(**inputs)
    key = (g_w, chunk_windows)
    if key not in _CACHE:
        _CACHE[key] = _build(np.array(g_w), [list(cw) for cw in chunk_windows])
    return in_maps, inv, _CACHE[key]


def kernel(**inputs):
    in_maps, inv, nc = _get(inputs)
    out, _ = _run(in_maps, inv, nc, trace=False)
    return out


def kernel_traced(**inputs):
    in_maps, inv, nc = _get(inputs)
    return _run(in_maps, inv, nc, trace=True)


# revision 9
# speedup vs baseline: 1.2317x; 1.1039x over previous
"""Trainium2 Bass kernel for EquivariantLieConvLayer (GNN message passing).

Math restructuring (exact algebra, not approximation):
  reference computes, per edge e = (s -> t):
      msg_e = alpha_bil * bracket(alpha_msg * F[s], F[t])
      agg[t] += msg_e
      out = F + agg + update_scale * bracket(agg, alpha_w * agg)
  * bracket is bilinear and F[t] is shared by all edges targeting t, so
      agg[t] = alpha_bil*alpha_msg * bracket(sum_{e->t} F[src_e], F[t])
    This removes the per-edge bracket entirely: only a scatter-add of raw
    source rows, then ONE bracket per node.
  * bracket(x, a*x) == 0 exactly (structure constants are antisymmetrized
    with zero diagonal), so the update bracket vanishes and
      out = F + agg.

Device mapping (8 NeuronCores, no collectives):
  Target nodes are assigned host-side to 160 (core, window) bins of <=128
  nodes, balancing per-bin in-edge counts so every bin needs the same
  number of 128-edge groups (SPMD-uniform instruction stream).  Per core:
    - dma_gather pulls bf16 source rows (padded to 256 cols) from a
      replicated DRAM feature table; gathered edges land 1/partition.
    - per window, one-hot matmuls (edges on K) accumulate
      S^T = sum of source rows, feature-major, in PSUM (f32).
    - bracket via factorized matmuls: Gx = Q^T S^T, Gy = Q^T F^T,
      terms+/- = GxA*GyB / GxB*GyA (DVE), agg = terms^T @ P (cv folded in P).
    - out = F(f32) + agg, DMA'd out node-major; host unpermutes rows.
"""

import numpy as np
import ml_dtypes

import concourse.bass as bass
import concourse.tile as tile
from concourse import bacc, mybir
from concourse.bass_utils import run_bass_kernel_spmd

BF16 = mybir.dt.bfloat16
F32 = mybir.dt.float32
I16 = mybir.dt.int16

N_NODES = 20000
D = 248
D_PAD = 256
N_CORES = 8
N_CPAD = 2560                     # padded node slots per core: 20 windows of 128
N_WIN = N_CPAD // 128             # 20
NB = 300                          # base structure-constant triples
TS = 384                          # padded per-side t dim (3 chunks of 128)
NODE_CHUNK = 256                  # bracket node chunk (2 windows)

_CACHE = {}


def _build(g_w, chunk_windows):
    """Build + compile the SPMD program. g_w[w] = #128-edge groups for window
    w (uniform across cores); chunk_windows = list of window-id lists per
    gather chunk."""
    tot_g = int(sum(g_w))
    g_off = np.concatenate([[0], np.cumsum(g_w)]).astype(int)

    nc = bacc.Bacc("TRN2", target_bir_lowering=False, debug=False,
                   num_devices=N_CORES)

    ftable = nc.dram_tensor("ftable", [N_NODES + 1, D_PAD], BF16, kind="ExternalInput")
    gidx = nc.dram_tensor("gidx", [128, tot_g * 8], I16, kind="ExternalInput")
    tgtcols = nc.dram_tensor("tgtcols", [128, tot_g], BF16, kind="ExternalInput")
    iotac = nc.dram_tensor("iotac", [128, 128], BF16, kind="ExternalInput")
    qmat = nc.dram_tensor("qmat", [D_PAD, 2 * TS], BF16, kind="ExternalInput")
    pmat = nc.dram_tensor("pmat", [2 * TS, D_PAD], BF16, kind="ExternalInput")
    ftr = nc.dram_tensor("ftr", [D_PAD, N_CPAD], BF16, kind="ExternalInput")
    fnode = nc.dram_tensor("fnode", [N_CPAD, D_PAD], F32, kind="ExternalInput")
    out_d = nc.dram_tensor("out", [N_CPAD, D_PAD], F32, kind="ExternalOutput")

    max_chunk_g = max(int(sum(g_w[w] for w in cw)) for cw in chunk_windows)
    n_nchunks = N_CPAD // NODE_CHUNK  # 5

    with tile.TileContext(nc) as tc:
        with tc.tile_pool(name="const", bufs=1) as cpool, \
             tc.tile_pool(name="gpool", bufs=3) as gpool, \
             tc.tile_pool(name="hpool", bufs=2) as hpool, \
             tc.tile_pool(name="work", bufs=2) as wpool, \
             tc.tile_pool(name="psum", bufs=1, space="PSUM") as pp:

            # ---- idx load first, then gathers ASAP (Pool engine = critical path)
            idx_sb = cpool.tile([128, tot_g * 8], I16, tag="idx")
            nc.sync.dma_start(out=idx_sb[:], in_=gidx.ap())

            g_tiles = []
            idx_pos = 0
            for cw in chunk_windows:
                cg = int(sum(g_w[w] for w in cw))
                n_idx = cg * 128
                g_t = gpool.tile([128, max_chunk_g, D_PAD], BF16, tag="G",
                                 name=f"G{len(g_tiles)}")
                nc.gpsimd.dma_gather(
                    out_ap=g_t[:, :cg, :],
                    in_ap=ftable.ap(),
                    idxs_ap=idx_sb[:, idx_pos // 16:(idx_pos + n_idx) // 16],
                    num_idxs=n_idx,
                    num_idxs_reg=n_idx,
                    elem_size=D_PAD,
                    single_packet=False,
                )
                g_tiles.append((g_t, idx_pos // 128))
                idx_pos += n_idx

            # ---- remaining constant loads ----
            tcol_sb = cpool.tile([128, tot_g], BF16, tag="tcol")
            nc.sync.dma_start(out=tcol_sb[:], in_=tgtcols.ap())
            iota_sb = cpool.tile([128, 128], BF16, tag="iota")
            nc.sync.dma_start(out=iota_sb[:], in_=iotac.ap())
            q_sb = [cpool.tile([128, 2 * TS], BF16, tag=f"q{h}", name=f"q{h}")
                    for h in range(2)]
            for h in range(2):
                nc.sync.dma_start(out=q_sb[h][:], in_=qmat.ap()[h * 128:(h + 1) * 128, :])
            p_sb = [cpool.tile([128, D_PAD], BF16, tag=f"p{m}", name=f"p{m}")
                    for m in range(6)]
            for m in range(6):
                nc.sync.dma_start(out=p_sb[m][:], in_=pmat.ap()[m * 128:(m + 1) * 128, :])
            ftr_sb = [cpool.tile([128, N_CPAD], BF16, tag=f"ftr{h}", name=f"ftr{h}")
                      for h in range(2)]
            for h in range(2):
                nc.sync.dma_start(out=ftr_sb[h][:], in_=ftr.ap()[h * 128:(h + 1) * 128, :])

            # ---- Gy = Q^T F^T (PE filler while gathers generate) ----
            gy_sb = [[None] * n_nchunks for _ in range(6)]
            for cn in range(n_nchunks):
                nsl = slice(cn * NODE_CHUNK, (cn + 1) * NODE_CHUNK)
                for m in range(6):
                    pt = pp.tile([128, NODE_CHUNK], F32, tag="gxy", bufs=2,
                                 name=f"gyp{cn}_{m}")
                    msl = slice(m * 128, (m + 1) * 128)
                    nc.tensor.matmul(out=pt[:], lhsT=q_sb[0][:, msl],
                                     rhs=ftr_sb[0][:, nsl], start=True, stop=False)
                    nc.tensor.matmul(out=pt[:], lhsT=q_sb[1][:, msl],
                                     rhs=ftr_sb[1][:, nsl], start=False, stop=True)
                    gt = wpool.tile([128, NODE_CHUNK], BF16, tag=f"gy{m}_{cn}",
                                    bufs=1, name=f"gy{m}_{cn}")
                    nc.vector.tensor_copy(out=gt[:], in_=pt[:])
                    gy_sb[m][cn] = gt

            # ---- scatter + bracket, interleaved per pair of windows ----
            sT = [cpool.tile([128, N_CPAD], BF16, tag=f"sT{h}", name=f"sT{h}")
                  for h in range(2)]
            win_chunk = {}
            for ci, cw in enumerate(chunk_windows):
                for w in cw:
                    win_chunk[w] = ci

            def scatter_window(w):
                gw = int(g_w[w])
                g_t, c_goff = g_tiles[win_chunk[w]]
                h_t = hpool.tile([128, gw * 128], BF16, tag="H", name=f"H{w}")
                in0 = bass.AP(iota_sb[:].tensor, iota_sb[:].offset,
                              [[128, 128], [0, gw], [1, 128]])
                tsl = tcol_sb[:, g_off[w]:g_off[w] + gw]
                in1 = bass.AP(tsl.tensor, tsl.offset,
                              [[tot_g, 128], [1, gw], [0, 128]])
                outap = bass.AP(h_t[:].tensor, h_t[:].offset,
                                [[gw * 128, 128], [128, gw], [1, 128]])
                nc.vector.tensor_tensor(out=outap, in0=in0, in1=in1,
                                        op=mybir.AluOpType.is_equal)
                ps = [pp.tile([128, 128], F32, tag="swin", bufs=4,
                              name=f"ps{w}_{hh}") for hh in range(2)]
                for g in range(gw):
                    slot = g_off[w] + g - c_goff
                    for h in range(2):
                        nc.tensor.matmul(
                            out=ps[h][:],
                            lhsT=g_t[:, slot, h * 128:(h + 1) * 128],
                            rhs=h_t[:, g * 128:(g + 1) * 128],
                            start=(g == 0), stop=(g == gw - 1),
                        )
                for h in range(2):
                    nc.vector.tensor_copy(
                        out=sT[h][:, w * 128:(w + 1) * 128], in_=ps[h][:])

            def bracket_chunk(cn):
                nsl = slice(cn * NODE_CHUNK, (cn + 1) * NODE_CHUNK)
                terms = [None] * 6
                for m in range(6):
                    pt = pp.tile([128, NODE_CHUNK], F32, tag="gxy", bufs=2,
                                 name=f"gxp{cn}_{m}")
                    msl = slice(m * 128, (m + 1) * 128)
                    nc.tensor.matmul(out=pt[:], lhsT=q_sb[0][:, msl],
                                     rhs=sT[0][:, nsl], start=True, stop=False)
                    nc.tensor.matmul(out=pt[:], lhsT=q_sb[1][:, msl],
                                     rhs=sT[1][:, nsl], start=False, stop=True)
                    tm = wpool.tile([128, NODE_CHUNK], BF16, tag=f"terms{m}",
                                    bufs=2, name=f"terms{m}_{cn}")
                    gy_other = gy_sb[m + 3][cn] if m < 3 else gy_sb[m - 3][cn]
                    nc.vector.tensor_tensor(out=tm[:], in0=pt[:], in1=gy_other[:],
                                            op=mybir.AluOpType.mult)
                    terms[m] = tm
                for nt in range(NODE_CHUNK // 128):
                    po = pp.tile([128, D_PAD], F32, tag="out", bufs=2,
                                 name=f"po{cn}_{nt}")
                    for m in range(6):
                        nc.tensor.matmul(out=po[:],
                                         lhsT=terms[m][:, nt * 128:(nt + 1) * 128],
                                         rhs=p_sb[m][:],
                                         start=(m == 0), stop=(m == 5))
                    r0 = cn * NODE_CHUNK + nt * 128
                    fnt = wpool.tile([128, D_PAD], F32, tag="fn", bufs=3,
                                     name=f"fn{cn}_{nt}")
                    nc.sync.dma_start(out=fnt[:], in_=fnode.ap()[r0:r0 + 128, :])
                    osb = wpool.tile([128, D_PAD], F32, tag="osb", bufs=3,
                                     name=f"osb{cn}_{nt}")
                    nc.vector.tensor_tensor(out=osb[:], in0=po[:], in1=fnt[:],
                                            op=mybir.AluOpType.add)
                    nc.sync.dma_start(out=out_d.ap()[r0:r0 + 128, :], in_=osb[:])

            wpc = NODE_CHUNK // 128   # windows per bracket chunk
            for w in range(N_WIN):
                scatter_window(w)
                if w % wpc == wpc - 1:
                    bracket_chunk(w // wpc)

    nc.compile()
    return nc


def _prep(features, edge_index, ci, cj, ck, cv,
          alpha_msg, alpha_bil, alpha_w, update_scale):
    F = np.asarray(features, np.float32)
    ei = np.asarray(edge_index)
    ci = np.asarray(ci); cj = np.asarray(cj); ck = np.asarray(ck)
    cv = np.asarray(cv, np.float32)
    am = float(alpha_msg); ab = float(alpha_bil)
    src, tgt = ei[0].astype(np.int64), ei[1].astype(np.int64)
    bf = ml_dtypes.bfloat16
    n_bins = N_CORES * N_WIN

    # --- balanced assignment of nodes to (core, window) bins ---
    deg = np.bincount(tgt, minlength=N_NODES)
    order = np.argsort(-deg, kind="stable")
    bin_load = np.zeros(n_bins, np.int64)
    bin_fill = np.zeros(n_bins, np.int64)
    node_bin = np.empty(N_NODES, np.int64)
    node_slot = np.empty(N_NODES, np.int64)
    import heapq
    heap = [(0, b) for b in range(n_bins)]
    heapq.heapify(heap)
    for n in order:
        while True:
            load, b = heapq.heappop(heap)
            if bin_fill[b] < 128:
                break
        node_bin[n] = b
        node_slot[n] = bin_fill[b]
        bin_fill[b] += 1
        bin_load[b] = load + deg[n]
        if bin_fill[b] < 128:
            heapq.heappush(heap, (bin_load[b], b))

    g_w_all = np.ceil(bin_load.reshape(N_CORES, N_WIN) / 128).astype(np.int64)
    g_w = np.maximum(1, g_w_all.max(axis=0))
    tot_g = int(g_w.sum())
    g_offs = np.concatenate([[0], np.cumsum(g_w)]).astype(int)

    # local (padded) node id within a core for each node
    node_core = node_bin // N_WIN
    node_win = node_bin % N_WIN
    node_local = node_win * 128 + node_slot          # in [0, 2560)

    # gather chunks: 2 windows per chunk, last two windows solo (short tail)
    chunk_windows = [[w, w + 1] for w in range(0, N_WIN - 2, 2)]
    chunk_windows += [[N_WIN - 2], [N_WIN - 1]]

    # --- per-core edge slots ---
    e_core = node_core[tgt]
    e_win = node_win[tgt]
    tot_idx = tot_g * 128
    idx_all = np.zeros((N_CORES, tot_idx), np.int16)
    col_all = np.full((N_CORES, tot_idx), -1.0, np.float32)
    eorder = np.lexsort((tgt, e_win, e_core))
    src_s = src[eorder]; core_s = e_core[eorder]; win_s = e_win[eorder]
    tl_s = (node_local[tgt] - node_win[tgt] * 128)[eorder]  # slot within window
    counts = np.zeros((N_CORES, N_WIN), np.int64)
    np.add.at(counts, (core_s, win_s), 1)
    run_starts = np.zeros((N_CORES, N_WIN), np.int64)
    np.cumsum(counts.ravel()[:-1], out=run_starts.ravel()[1:])
    for c in range(N_CORES):
        for w in range(N_WIN):
            cnt = int(counts[c, w]); s0 = int(run_starts[c, w])
            base = g_offs[w] * 128
            idx_all[c, base:base + cnt] = src_s[s0:s0 + cnt].astype(np.int16)
            col_all[c, base:base + cnt] = tl_s[s0:s0 + cnt].astype(np.float32)

    # --- constant tables ---
    ftable = np.zeros((N_NODES + 1, D_PAD), bf)
    ftable[:N_NODES, :D] = F.astype(bf)
    iota = np.broadcast_to(np.arange(128, dtype=np.float32), (128, 128)).astype(bf)
    Q = np.zeros((D_PAD, 2 * TS), np.float32)
    i_s, j_s, k_s, v_s = ci[:NB], cj[:NB], ck[:NB], cv[:NB]
    Q[i_s, np.arange(NB)] = 1.0
    Q[j_s, TS + np.arange(NB)] = 1.0
    scale = ab * am
    P = np.zeros((2 * TS, D_PAD), np.float32)
    P[np.arange(NB), k_s] = v_s * scale
    P[TS + np.arange(NB), k_s] = -v_s * scale

    # permuted F slices per core
    in_maps = []
    # inverse map: (core, local) -> original node (or -1)
    inv = np.full((N_CORES, N_CPAD), -1, np.int64)
    inv[node_core, node_local] = np.arange(N_NODES)
    for c in range(N_CORES):
        wrapped = idx_all[c].reshape(tot_idx // 16, 16).T
        gidx = np.tile(wrapped, (8, 1)).copy()
        tcols = col_all[c].reshape(tot_g, 128).T.astype(bf).copy()
        sel = inv[c]
        valid = sel >= 0
        fslice = np.zeros((N_CPAD, D_PAD), np.float32)
        fslice[valid, :D] = F[sel[valid]]
        ftr_c = np.zeros((D_PAD, N_CPAD), bf)
        ftr_c[:D, valid] = F[sel[valid]].T.astype(bf)
        in_maps.append({
            "ftable": ftable,
            "gidx": gidx,
            "tgtcols": tcols,
            "iotac": iota,
            "qmat": Q.astype(bf),
            "pmat": P.astype(bf),
            "ftr": ftr_c,
            "fnode": fslice,
        })
    return (tuple(g_w.tolist()), tuple(tuple(cw) for cw in chunk_windows),
            in_maps, inv)


def _run(in_maps, inv, nc, trace=False):
    res = run_bass_kernel_spmd(nc, in_maps, core_ids=list(range(N_CORES)),
                               trace=trace)
    out = np.empty((N_NODES, D), np.float32)
    for c in range(N_CORES):
        sel = inv[c]
        valid = sel >= 0
        out[sel[valid]] = res.results[c]["out"][valid, :D]
    return out, res


def _get(inputs):
    g_w, chunk_windows, in_maps, inv = _prep(**inputs)
    key = (g_w, chunk_windows)
    if key not in _CACHE:
        _CACHE[key] = _build(np.array(g_w), [list(cw) for cw in chunk_windows])
    return in_maps, inv, _CACHE[key]


def kernel(**inputs):
    in_maps, inv, nc = _get(inputs)
    out, _ = _run(in_maps, inv, nc, trace=False)
    return out


def kernel_traced(**inputs):
    in_maps, inv, nc = _get(inputs)
    return _run(in_maps, inv, nc, trace=True)
